# revision 25
# baseline (speedup 1.0000x reference)
"""2-layer GCN forward on 8 Trainium2 NeuronCores (Bass/Tile), v2.

Reformulation: out_l = (A_n @ u) @ W + b with A_n = D^-1/2 A_w D^-1/2
(incl. self loops).  Since A_n @ (x W1) = (A_n x) W1, layer 1 gathers
RAW x rows (available at t=0; no replicated u1 phase) and applies W1
per dest block after the scatter-add.

All per-edge normalization (dinv_src * w * dinv_dst) is folded on the
HOST into dense per-chunk scatter matrices S [128 msgs, 128 dests],
streamed from DRAM over the otherwise-idle HWDGE path.  This removes
every on-device one-hot build (the old DVE bottleneck) and the
deg/dinv computation + deg AllGather.

Per dest block: chunk 0 is the "self chunk" whose messages are the
block's own 128 rows (self loops + intra-block edges + their dups),
streamed sequentially via HWDGE -- no SWDGE descriptors.  Remaining
edges are deduped by (block, src) and packed into variable per-block
chunk counts (max over the 8 cores, not global max).  SWDGE dma_gather
(4 queues) pulls the 256B rows; int16 reach handled by a lo/hi table
split at 32768.

u2 = h1 @ W2 is written per block during L1 and exchanged with one
AllGather per supergroup so the collective pipelines behind L1 compute
instead of being a barrier.  The collective requires contiguous
outputs, so u2_tab uses a supergroup-major row permutation; the L2
gather uses its own host-built chunk tables in permuted row space.
"""

import math
import os

import numpy as np

import concourse.bacc as bacc
import concourse.bass as bass
import concourse.mybir as mybir
import concourse.tile as tile
from concourse.bass_utils import run_bass_kernel_spmd

P = 128
NCORES = 8
SG = 5  # dest blocks per gather supergroup
LO_LIMIT = 32768  # int16 index reach for dma_gather

F32 = mybir.dt.float32
F16 = mybir.dt.float16
I16 = mybir.dt.int16

_last_results = {}


def _wrap_idx(arr):
    """int16 stream -> [128, len/16] wrapped layout for dma_gather."""
    assert len(arr) % 16 == 0
    a = arr.reshape(-1, 16).T  # [16, len/16]
    return np.ascontiguousarray(np.tile(a, (8, 1)))  # [128, len/16]


def _chunk_tables(prow, r_dst, r_norm, i_src, i_dst, i_norm,
                  NPAD, NBLK, HI_BASE):
    """Build per-core chunk tables for one gather space.

    prow: permuted gather-table row per regular edge's src.
    Returns cfg dict + per-core list of (S_T, idx_lo_w, idx_hi_w).
    """
    NB_ALL = NPAD // P
    r_blk = r_dst // P
    r_half = (prow >= LO_LIMIT).astype(np.int64)
    key = (r_blk * 2 + r_half) * NPAD + prow
    order = np.argsort(key, kind="stable")
    ks = key[order]
    newgrp = np.r_[True, ks[1:] != ks[:-1]]
    uid_of_sorted = np.cumsum(newgrp) - 1
    uid = np.empty(len(ks), np.int64)
    uid[order] = uid_of_sorted
    u_key = ks[newgrp]
    u_row = prow[order][newgrp]
    u_g = u_key // NPAD
    grp_start = np.searchsorted(u_g, np.arange(NB_ALL * 2 + 1))
    u_rank = np.arange(len(u_row)) - grp_start[u_g]
    cnt = np.diff(grp_start).reshape(NB_ALL, 2)

    cpc = cnt.reshape(NCORES, NBLK, 2)
    CH_LO = np.ceil(cpc[:, :, 0].max(axis=0) / P).astype(np.int64)
    CH_HI = np.ceil(cpc[:, :, 1].max(axis=0) / P).astype(np.int64)
    lo_off = np.concatenate([[0], np.cumsum(CH_LO)])
    hi_off = np.concatenate([[0], np.cumsum(CH_HI)])
    stot = 1 + CH_LO + CH_HI
    soff = np.concatenate([[0], np.cumsum(stot)])
    SCHT = int(soff[-1])
    NLO = int(lo_off[-1])
    NHI = int(hi_off[-1])

    u_blk = u_g // 2
    u_half = u_g % 2
    u_k = u_blk % NBLK
    u_core = u_blk // NBLK
    u_cih = u_rank // P
    u_slot = u_rank % P
    u_schunk = soff[u_k] + 1 + np.where(u_half == 0, u_cih,
                                        CH_LO[u_k] + u_cih)
    u_idxpos = np.where(u_half == 0,
                        (lo_off[u_k] + u_cih) * P + u_slot,
                        (hi_off[u_k] + u_cih) * P + u_slot)

    e_core = u_core[uid]
    e_flat = (u_schunk[uid] * P + u_slot[uid]) * P + (r_dst % P)
    i_blk = i_dst // P
    i_core = i_blk // NBLK
    i_flat = (soff[i_blk % NBLK] * P + (i_src % P)) * P + (i_dst % P)

    per_core = []
    for i in range(NCORES):
        S = np.zeros(SCHT * P * P, np.float32)
        m = e_core == i
        np.add.at(S, e_flat[m], r_norm[m])
        m = i_core == i
        np.add.at(S, i_flat[m], i_norm[m])
        S_T = np.ascontiguousarray(
            S.reshape(SCHT, P, P).astype(np.float16)
            .transpose(1, 0, 2).reshape(P, SCHT * P))

        idx_lo = np.zeros(max(NLO, 1) * P, np.int16)
        idx_hi = np.zeros(max(NHI, 1) * P, np.int16)
        m = u_core == i
        mlo = m & (u_half == 0)
        mhi = m & (u_half == 1)
        idx_lo[u_idxpos[mlo]] = u_row[mlo].astype(np.int16)
        idx_hi[u_idxpos[mhi]] = (u_row[mhi] - HI_BASE).astype(np.int16)
        per_core.append((S_T, _wrap_idx(idx_lo), _wrap_idx(idx_hi)))

    cfg = dict(CH_LO=CH_LO.tolist(), CH_HI=CH_HI.tolist(),
               lo_off=lo_off.tolist(), hi_off=hi_off.tolist(),
               soff=soff.tolist(), SCHT=SCHT, NLO=NLO, NHI=NHI)
    return cfg, per_core


def _prep(x, edge_index, edge_weight, W1, b1, W2, b2):
    N, F = x.shape
    H = W1.shape[1]
    C = W2.shape[1]
    assert F == 128 and H == 128 and C == 64, (F, H, C)

    NPC = ((N + NCORES * P - 1) // (NCORES * P)) * P
    NPAD = NPC * NCORES
    NBLK = NPC // P
    HI_BASE = NPAD - LO_LIMIT
    assert 0 < HI_BASE <= LO_LIMIT

    src = np.asarray(edge_index[0], dtype=np.int64)
    dst = np.asarray(edge_index[1], dtype=np.int64)
    w = np.asarray(edge_weight, dtype=np.float64)
    loop = np.arange(N, dtype=np.int64)
    src_a = np.concatenate([src, loop])
    dst_a = np.concatenate([dst, loop])
    w_a = np.concatenate([w, np.ones(N, np.float64)])

    deg = np.zeros(NPAD, np.float64)
    np.add.at(deg, dst_a, w_a)
    dinv = np.where(deg > 0, 1.0 / np.sqrt(np.maximum(deg, 1e-30)), 0.0)
    norm = (dinv[src_a] * w_a * dinv[dst_a]).astype(np.float32)

    blk = dst_a // P
    intra = (src_a // P) == blk
    r_src = src_a[~intra]
    r_dst = dst_a[~intra]
    r_norm = norm[~intra]
    i_src = src_a[intra]
    i_dst = dst_a[intra]
    i_norm = norm[intra]

    # AG-range-major row permutation for u2_tab: each partial AllGather
    # writes its 8 cores' contributions contiguously, so the permutation
    # must be interleaved per AG range (not per gather supergroup).
    sgs = [(s, min(s + SG, NBLK)) for s in range(0, NBLK, SG)]
    ag_ranges = [(0, 15), (15, 30), (30, 45), (45, NBLK)]
    pblock = np.empty(NPAD // P, np.int64)
    for r0, r1 in ag_ranges:
        nr = r1 - r0
        for c in range(NCORES):
            for k in range(r0, r1):
                pblock[c * NBLK + k] = 8 * r0 + c * nr + (k - r0)
    node = np.arange(NPAD)
    perm_row = pblock[node // P] * P + node % P  # natural node -> u2_tab row

    cfg1, tabs1 = _chunk_tables(r_src, r_dst, r_norm, i_src, i_dst, i_norm,
                                NPAD, NBLK, HI_BASE)
    cfg2, tabs2 = _chunk_tables(perm_row[r_src], r_dst, r_norm,
                                i_src, i_dst, i_norm, NPAD, NBLK, HI_BASE)

    xtab = np.zeros((NPAD, P), np.float16)
    xtab[:N] = np.asarray(x, np.float32).astype(np.float16)
    common = {
        "xtab": xtab,
        "W1": np.asarray(W1, np.float32).astype(np.float16),
        "W2": np.asarray(W2, np.float32).astype(np.float16),
        "b1c": np.asarray(b1, np.float32).reshape(P, 1),
        "b2r": np.asarray(b2, np.float32).astype(np.float16)[None, :],
        "onesr": np.ones((1, P), np.float16),
    }

    in_maps = []
    for i in range(NCORES):
        d = {
            "S1_T": tabs1[i][0], "idx1_lo": tabs1[i][1],
            "idx1_hi": tabs1[i][2],
            "S2_T": tabs2[i][0], "idx2_lo": tabs2[i][1],
            "idx2_hi": tabs2[i][2],
            "xloc": np.ascontiguousarray(xtab[i * NPC:(i + 1) * NPC]),
        }
        d.update(common)
        in_maps.append(d)

    cfg = dict(N=N, NPC=NPC, NPAD=NPAD, NBLK=NBLK, HI_BASE=HI_BASE,
               H=H, C=C, sgs=sgs, ag_ranges=ag_ranges, L1=cfg1, L2=cfg2)
    return in_maps, cfg


def _split_gather(nc, qn, gtile, src, idx_tile, ch0, nch, elem):
    """Issue a gather as two half-gathers on different SWDGE queues."""
    h1 = (nch + 1) // 2
    for lo, hi in ((0, h1), (h1, nch)):
        if hi <= lo:
            continue
        ni = (hi - lo) * P
        nc.gpsimd.dma_gather(
            gtile[:, lo:hi, :], src,
            idx_tile[:, (ch0 + lo) * 8:(ch0 + hi) * 8],
            ni, ni, elem, single_packet=False, queue_num=qn(0))


def _build(cfg):
    NPC, NPAD, NBLK = cfg["NPC"], cfg["NPAD"], cfg["NBLK"]
    HI_BASE, H, C = cfg["HI_BASE"], cfg["H"], cfg["C"]
    sgs = cfg["sgs"]
    L1, L2 = cfg["L1"], cfg["L2"]
    AF = mybir.ActivationFunctionType
    AL = mybir.AluOpType

    nc = bacc.Bacc("TRN2", target_bir_lowering=False, debug=False,
                   num_devices=NCORES, num_swdge_queues=4)

    xtab_d = nc.dram_tensor("xtab", [NPAD, P], F16, kind="ExternalInput")
    xloc_d = nc.dram_tensor("xloc", [NPC, P], F16, kind="ExternalInput")
    S1_d = nc.dram_tensor("S1_T", [P, L1["SCHT"] * P], F16,
                          kind="ExternalInput")
    S2_d = nc.dram_tensor("S2_T", [P, L2["SCHT"] * P], F16,
                          kind="ExternalInput")
    W1_d = nc.dram_tensor("W1", [P, H], F16, kind="ExternalInput")
    W2_d = nc.dram_tensor("W2", [P, C], F16, kind="ExternalInput")
    b1_d = nc.dram_tensor("b1c", [P, 1], F32, kind="ExternalInput")
    b2_d = nc.dram_tensor("b2r", [1, C], F16, kind="ExternalInput")
    on_d = nc.dram_tensor("onesr", [1, P], F16, kind="ExternalInput")
    i1l_d = nc.dram_tensor("idx1_lo", [P, max(L1["NLO"], 1) * 8], I16,
                           kind="ExternalInput")
    i1h_d = nc.dram_tensor("idx1_hi", [P, max(L1["NHI"], 1) * 8], I16,
                           kind="ExternalInput")
    i2l_d = nc.dram_tensor("idx2_lo", [P, max(L2["NLO"], 1) * 8], I16,
                           kind="ExternalInput")
    i2h_d = nc.dram_tensor("idx2_hi", [P, max(L2["NHI"], 1) * 8], I16,
                           kind="ExternalInput")
    out_d = nc.dram_tensor("out", [NPC, C], F32, kind="ExternalOutput")

    u2_own = nc.dram_tensor("u2_own", [NPC, P], F16)
    u2_tab = nc.dram_tensor("u2_tab", [NPAD, P], F16, addr_space="Shared")

    rg = [list(range(NCORES))]
    _q = [0]

    def qn(_):
        _q[0] = (_q[0] + 1) % 4
        return _q[0]

    with tile.TileContext(nc) as tc:
        with (
            tc.tile_pool(name="const", bufs=1) as cp,
            tc.tile_pool(name="work", bufs=2) as wp,
            tc.tile_pool(name="psum", bufs=2, space="PSUM") as pp,
        ):
            # ---- constants (idx tables first: gathers gate on them) ----
            i1l = cp.tile([P, max(L1["NLO"], 1) * 8], I16)
            nc.sync.dma_start(i1l[:], i1l_d[:, :])
            i1h = cp.tile([P, max(L1["NHI"], 1) * 8], I16)
            nc.sync.dma_start(i1h[:], i1h_d[:, :])
            W1s = cp.tile([P, H], F16)
            nc.sync.dma_start(W1s[:], W1_d[:, :])
            W2s = cp.tile([P, C], F16)
            nc.sync.dma_start(W2s[:], W2_d[:, :])
            b1s = cp.tile([P, 1], F32)
            nc.sync.dma_start(b1s[:], b1_d[:, :])
            b2s = cp.tile([1, C], F16)
            nc.sync.dma_start(b2s[:], b2_d[:, :])
            ones = cp.tile([1, P], F16)
            nc.sync.dma_start(ones[:], on_d[:, :])

            x_lo = xtab_d[0:LO_LIMIT, :]
            x_hi = xtab_d[HI_BASE:NPAD, :]

            # ---- layer 1: scatter raw x, then W1 / relu / W2 per block ----
            CH_LO, CH_HI = L1["CH_LO"], L1["CH_HI"]
            lo_off, hi_off, soff = L1["lo_off"], L1["hi_off"], L1["soff"]

            def emit_ag(b0, b1_):
                # partial AllGather of a block range's u2 rows into the
                # supergroup-major (contiguous-output) u2_tab layout; the
                # range must cover whole supergroups.
                nsg = b1_ - b0
                go = 8 * b0 * P
                nc.gpsimd.collective_compute(
                    "AllGather", AL.bypass, replica_groups=rg,
                    ins=[u2_own.ap()[b0 * P:b1_ * P, :]],
                    outs=[u2_tab.ap()[go:go + 8 * nsg * P, :]])

            # AG block ranges (whole supergroups) and the sg index at whose
            # loop-top they are emitted: two supergroups after the range
            # completes, so the trigger's wait never stalls gather issue.
            # emit each AG two supergroups after its range completes so the
            # trigger's wait is already satisfied when it reaches the
            # GpSimd queue head (ranges complete at sgs 2, 5, 8, 9).
            agr = cfg["ag_ranges"]
            ag_plan = {4: agr[0], 7: agr[1], 9: agr[2]}  # sg-index -> range

            for i_sg, (b0, b1_) in enumerate(sgs):
                nlo = lo_off[b1_] - lo_off[b0]
                nhi = hi_off[b1_] - hi_off[b0]
                nst = soff[b1_] - soff[b0]
                if nlo:
                    glo = wp.tile([P, nlo, P], F16, tag="glo", bufs=3)
                    _split_gather(nc, qn, glo, x_lo, i1l, lo_off[b0], nlo, H)
                if nhi:
                    ghi = wp.tile([P, nhi, P], F16, tag="ghi", bufs=3)
                    _split_gather(nc, qn, ghi, x_hi, i1h, hi_off[b0], nhi, H)
                nsg = b1_ - b0
                xsf = wp.tile([P, nsg, P], F16, tag="xsf", bufs=2)
                nc.sync.dma_start(
                    xsf[:],
                    xloc_d.ap().rearrange("(k p) f -> p k f",
                                          p=P)[:, b0:b1_, :])
                sst = wp.tile([P, nst * P], F16, tag="sst", bufs=3)
                nc.sync.dma_start(sst[:],
                                  S1_d[:, soff[b0] * P:soff[b1_] * P])
                if i_sg in ag_plan:
                    emit_ag(*ag_plan[i_sg])
                for b in range(b0, b1_):
                    sb = (soff[b] - soff[b0]) * P
                    ph = pp.tile([P, P], F32, tag="ph")
                    nc.tensor.matmul(ph[:], xsf[:, b - b0, :],
                                     sst[:, sb:sb + P],
                                     start=True, stop=False)
                    nch = CH_LO[b] + CH_HI[b]
                    for j in range(CH_LO[b]):
                        c = sb + (1 + j) * P
                        g = lo_off[b] - lo_off[b0] + j
                        nc.tensor.matmul(ph[:], glo[:, g, :],
                                         sst[:, c:c + P],
                                         start=False, stop=(j == nch - 1))
                    for j in range(CH_HI[b]):
                        c = sb + (1 + CH_LO[b] + j) * P
                        g = hi_off[b] - hi_off[b0] + j
                        nc.tensor.matmul(ph[:], ghi[:, g, :],
                                         sst[:, c:c + P],
                                         start=False,
                                         stop=(CH_LO[b] + j == nch - 1))
                    g1T = wp.tile([P, P], F16, tag="g1T")
                    nc.vector.tensor_copy(g1T[:], ph[:])
                    ph2 = pp.tile([P, P], F32, tag="ph2")
                    nc.tensor.matmul(ph2[:], W1s[:], g1T[:],
                                     start=True, stop=True)
                    h1T = wp.tile([P, P], F16, tag="h1T")
                    nc.scalar.activation(h1T[:], ph2[:], AF.Relu,
                                         bias=b1s[:, 0:1], scale=1.0)
                    pu2 = pp.tile([P, C], F32, tag="pu2")
                    nc.tensor.matmul(pu2[:], h1T[:], W2s[:],
                                     start=True, stop=True)
                    u2b = wp.tile([P, C], F16, tag="u2b")
                    nc.vector.tensor_copy(u2b[:], pu2[:])
                    nc.sync.dma_start(u2_own[b * P:(b + 1) * P, 0:C], u2b[:])
                qn(0)  # rotate queue mapping so lo/hi loads balance
            emit_ag(*agr[3])

            # ---- layer 2: scatter u2 rows, + b2 ----
            i2l = cp.tile([P, max(L2["NLO"], 1) * 8], I16)
            nc.sync.dma_start(i2l[:], i2l_d[:, :])
            i2h = cp.tile([P, max(L2["NHI"], 1) * 8], I16)
            nc.sync.dma_start(i2h[:], i2h_d[:, :])
            u_lo = u2_tab[0:LO_LIMIT, :]
            u_hi = u2_tab[HI_BASE:NPAD, :]
            CH_LO, CH_HI = L2["CH_LO"], L2["CH_HI"]
            lo_off, hi_off, soff = L2["lo_off"], L2["hi_off"], L2["soff"]
            for b0, b1_ in sgs:
                nlo = lo_off[b1_] - lo_off[b0]
                nhi = hi_off[b1_] - hi_off[b0]
                nst = soff[b1_] - soff[b0]
                if nlo:
                    glo = wp.tile([P, nlo, P], F16, tag="glo", bufs=3)
                    _split_gather(nc, qn, glo, u_lo, i2l, lo_off[b0], nlo, H)
                if nhi:
                    ghi = wp.tile([P, nhi, P], F16, tag="ghi", bufs=3)
                    _split_gather(nc, qn, ghi, u_hi, i2h, hi_off[b0], nhi, H)
                nsg = b1_ - b0
                usf = wp.tile([P, nsg, C], F16, tag="usf", bufs=2)
                nc.sync.dma_start(
                    usf[:],
                    u2_own.ap().rearrange("(k p) f -> p k f",
                                          p=P)[:, b0:b1_, 0:C])
                sst = wp.tile([P, nst * P], F16, tag="sst", bufs=3)
                nc.sync.dma_start(sst[:],
                                  S2_d[:, soff[b0] * P:soff[b1_] * P])
                for b in range(b0, b1_):
                    sb = (soff[b] - soff[b0]) * P
                    po = pp.tile([P, C], F32, tag="po")
                    nc.tensor.matmul(po[:], sst[:, sb:sb + P],
                                     usf[:, b - b0, :],
                                     start=True, stop=False)
                    for j in range(CH_LO[b]):
                        c = sb + (1 + j) * P
                        g = lo_off[b] - lo_off[b0] + j
                        nc.tensor.matmul(po[:], sst[:, c:c + P],
                                         glo[:, g, 0:C],
                                         start=False, stop=False)
                    for j in range(CH_HI[b]):
                        c = sb + (1 + CH_LO[b] + j) * P
                        g = hi_off[b] - hi_off[b0] + j
                        nc.tensor.matmul(po[:], sst[:, c:c + P],
                                         ghi[:, g, 0:C],
                                         start=False, stop=False)
                    nc.tensor.matmul(po[:], ones[:], b2s[:],
                                     start=False, stop=True)
                    ob = wp.tile([P, C], F32, tag="ob")
                    nc.vector.tensor_copy(ob[:], po[:])
                    nc.sync.dma_start(out_d[b * P:(b + 1) * P, :], ob[:])
                qn(0)  # rotate queue mapping so lo/hi loads balance

    nc.compile()
    return nc


def kernel(x, edge_index, edge_weight, W1, b1, W2, b2):
    in_maps, cfg = _prep(x, edge_index, edge_weight, W1, b1, W2, b2)
    nc = _build(cfg)
    trace = os.environ.get("GCN_TRACE", "0") == "1"
    res = run_bass_kernel_spmd(nc, in_maps, core_ids=list(range(NCORES)),
                               trace=trace)
    _last_results["exec_time_ns"] = res.exec_time_ns
    _last_results["results"] = res
    out = np.concatenate([r["out"] for r in res.results], axis=0)
    return np.ascontiguousarray(out[:cfg["N"]])


# revision 28
# speedup vs baseline: 1.0313x; 1.0313x over previous
"""2-layer GCN forward on 8 Trainium2 NeuronCores (Bass/Tile), v2.

Reformulation: out_l = (A_n @ u) @ W + b with A_n = D^-1/2 A_w D^-1/2
(incl. self loops).  Since A_n @ (x W1) = (A_n x) W1, layer 1 gathers
RAW x rows (available at t=0; no replicated u1 phase) and applies W1
per dest block after the scatter-add.

All per-edge normalization (dinv_src * w * dinv_dst) is folded on the
HOST into dense per-chunk scatter matrices S [128 msgs, 128 dests],
streamed from DRAM over the otherwise-idle HWDGE path.  This removes
every on-device one-hot build (the old DVE bottleneck) and the
deg/dinv computation + deg AllGather.

Per dest block: chunk 0 is the "self chunk" whose messages are the
block's own 128 rows (self loops + intra-block edges + their dups),
streamed sequentially via HWDGE -- no SWDGE descriptors.  Remaining
edges are deduped by (block, src) and packed into variable per-block
chunk counts (max over the 8 cores, not global max).  SWDGE dma_gather
(4 queues) pulls the 256B rows; int16 reach handled by a lo/hi table
split at 32768.

u2 = h1 @ W2 is written per block during L1 and exchanged with one
AllGather per supergroup so the collective pipelines behind L1 compute
instead of being a barrier.  The collective requires contiguous
outputs, so u2_tab uses a supergroup-major row permutation; the L2
gather uses its own host-built chunk tables in permuted row space.
"""

import math
import os

import numpy as np

import concourse.bacc as bacc
import concourse.bass as bass
import concourse.mybir as mybir
import concourse.tile as tile
from concourse.bass_utils import run_bass_kernel_spmd

P = 128
NCORES = 8
SG = 5  # dest blocks per gather supergroup
LO_LIMIT = 32768  # int16 index reach for dma_gather

F32 = mybir.dt.float32
F16 = mybir.dt.float16
I16 = mybir.dt.int16

_last_results = {}


def _wrap_idx(arr):
    """int16 stream -> [128, len/16] wrapped layout for dma_gather."""
    assert len(arr) % 16 == 0
    a = arr.reshape(-1, 16).T  # [16, len/16]
    return np.ascontiguousarray(np.tile(a, (8, 1)))  # [128, len/16]


def _chunk_tables(prow, r_dst, r_norm, i_src, i_dst, i_norm,
                  NPAD, NBLK, HI_BASE):
    """Build per-core chunk tables for one gather space.

    prow: permuted gather-table row per regular edge's src.
    Returns cfg dict + per-core list of (S_T, idx_lo_w, idx_hi_w).
    """
    NB_ALL = NPAD // P
    r_blk = r_dst // P
    r_half = (prow >= LO_LIMIT).astype(np.int64)
    key = (r_blk * 2 + r_half) * NPAD + prow
    order = np.argsort(key, kind="stable")
    ks = key[order]
    newgrp = np.r_[True, ks[1:] != ks[:-1]]
    uid_of_sorted = np.cumsum(newgrp) - 1
    uid = np.empty(len(ks), np.int64)
    uid[order] = uid_of_sorted
    u_key = ks[newgrp]
    u_row = prow[order][newgrp]
    u_g = u_key // NPAD
    grp_start = np.searchsorted(u_g, np.arange(NB_ALL * 2 + 1))
    u_rank = np.arange(len(u_row)) - grp_start[u_g]
    cnt = np.diff(grp_start).reshape(NB_ALL, 2)

    cpc = cnt.reshape(NCORES, NBLK, 2)
    CH_LO = np.ceil(cpc[:, :, 0].max(axis=0) / P).astype(np.int64)
    CH_HI = np.ceil(cpc[:, :, 1].max(axis=0) / P).astype(np.int64)
    lo_off = np.concatenate([[0], np.cumsum(CH_LO)])
    hi_off = np.concatenate([[0], np.cumsum(CH_HI)])
    stot = 1 + CH_LO + CH_HI
    soff = np.concatenate([[0], np.cumsum(stot)])
    SCHT = int(soff[-1])
    NLO = int(lo_off[-1])
    NHI = int(hi_off[-1])

    u_blk = u_g // 2
    u_half = u_g % 2
    u_k = u_blk % NBLK
    u_core = u_blk // NBLK
    u_cih = u_rank // P
    u_slot = u_rank % P
    u_schunk = soff[u_k] + 1 + np.where(u_half == 0, u_cih,
                                        CH_LO[u_k] + u_cih)
    u_idxpos = np.where(u_half == 0,
                        (lo_off[u_k] + u_cih) * P + u_slot,
                        (hi_off[u_k] + u_cih) * P + u_slot)

    e_core = u_core[uid]
    e_flat = (u_schunk[uid] * P + u_slot[uid]) * P + (r_dst % P)
    i_blk = i_dst // P
    i_core = i_blk // NBLK
    i_flat = (soff[i_blk % NBLK] * P + (i_src % P)) * P + (i_dst % P)

    per_core = []
    for i in range(NCORES):
        S = np.zeros(SCHT * P * P, np.float32)
        m = e_core == i
        np.add.at(S, e_flat[m], r_norm[m])
        m = i_core == i
        np.add.at(S, i_flat[m], i_norm[m])
        S_T = np.ascontiguousarray(
            S.reshape(SCHT, P, P).astype(np.float16)
            .transpose(1, 0, 2).reshape(P, SCHT * P))

        idx_lo = np.zeros(max(NLO, 1) * P, np.int16)
        idx_hi = np.zeros(max(NHI, 1) * P, np.int16)
        m = u_core == i
        mlo = m & (u_half == 0)
        mhi = m & (u_half == 1)
        idx_lo[u_idxpos[mlo]] = u_row[mlo].astype(np.int16)
        idx_hi[u_idxpos[mhi]] = (u_row[mhi] - HI_BASE).astype(np.int16)
        per_core.append((S_T, _wrap_idx(idx_lo), _wrap_idx(idx_hi)))

    cfg = dict(CH_LO=CH_LO.tolist(), CH_HI=CH_HI.tolist(),
               lo_off=lo_off.tolist(), hi_off=hi_off.tolist(),
               soff=soff.tolist(), SCHT=SCHT, NLO=NLO, NHI=NHI)
    return cfg, per_core


def _prep(x, edge_index, edge_weight, W1, b1, W2, b2):
    N, F = x.shape
    H = W1.shape[1]
    C = W2.shape[1]
    assert F == 128 and H == 128 and C == 64, (F, H, C)

    NPC = ((N + NCORES * P - 1) // (NCORES * P)) * P
    NPAD = NPC * NCORES
    NBLK = NPC // P
    HI_BASE = NPAD - LO_LIMIT
    assert 0 < HI_BASE <= LO_LIMIT

    src = np.asarray(edge_index[0], dtype=np.int64)
    dst = np.asarray(edge_index[1], dtype=np.int64)
    w = np.asarray(edge_weight, dtype=np.float64)
    loop = np.arange(N, dtype=np.int64)
    src_a = np.concatenate([src, loop])
    dst_a = np.concatenate([dst, loop])
    w_a = np.concatenate([w, np.ones(N, np.float64)])

    deg = np.zeros(NPAD, np.float64)
    np.add.at(deg, dst_a, w_a)
    dinv = np.where(deg > 0, 1.0 / np.sqrt(np.maximum(deg, 1e-30)), 0.0)
    norm = (dinv[src_a] * w_a * dinv[dst_a]).astype(np.float32)

    blk = dst_a // P
    intra = (src_a // P) == blk
    r_src = src_a[~intra]
    r_dst = dst_a[~intra]
    r_norm = norm[~intra]
    i_src = src_a[intra]
    i_dst = dst_a[intra]
    i_norm = norm[intra]

    # AG-range-major row permutation for u2_tab: each partial AllGather
    # writes its 8 cores' contributions contiguously, so the permutation
    # must be interleaved per AG range (not per gather supergroup).
    sgs = [(s, min(s + SG, NBLK)) for s in range(0, NBLK, SG)]
    ag_ranges = [(0, 15), (15, 30), (30, 45), (45, NBLK)]
    pblock = np.empty(NPAD // P, np.int64)
    for r0, r1 in ag_ranges:
        nr = r1 - r0
        for c in range(NCORES):
            for k in range(r0, r1):
                pblock[c * NBLK + k] = 8 * r0 + c * nr + (k - r0)
    node = np.arange(NPAD)
    perm_row = pblock[node // P] * P + node % P  # natural node -> u2_tab row

    cfg1, tabs1 = _chunk_tables(r_src, r_dst, r_norm, i_src, i_dst, i_norm,
                                NPAD, NBLK, HI_BASE)
    cfg2, tabs2 = _chunk_tables(perm_row[r_src], r_dst, r_norm,
                                i_src, i_dst, i_norm, NPAD, NBLK, HI_BASE)

    xtab = np.zeros((NPAD, P), np.float16)
    xtab[:N] = np.asarray(x, np.float32).astype(np.float16)
    common = {
        "xtab": xtab,
        "W1": np.asarray(W1, np.float32).astype(np.float16),
        "W2": np.asarray(W2, np.float32).astype(np.float16),
        "b1c": np.asarray(b1, np.float32).reshape(P, 1),
        "b2r": np.asarray(b2, np.float32).astype(np.float16)[None, :],
        "onesr": np.ones((1, P), np.float16),
    }

    in_maps = []
    for i in range(NCORES):
        d = {
            "S1_T": tabs1[i][0], "idx1_lo": tabs1[i][1],
            "idx1_hi": tabs1[i][2],
            "S2_T": tabs2[i][0], "idx2_lo": tabs2[i][1],
            "idx2_hi": tabs2[i][2],
            "xloc": np.ascontiguousarray(xtab[i * NPC:(i + 1) * NPC]),
        }
        d.update(common)
        in_maps.append(d)

    cfg = dict(N=N, NPC=NPC, NPAD=NPAD, NBLK=NBLK, HI_BASE=HI_BASE,
               H=H, C=C, sgs=sgs, ag_ranges=ag_ranges, L1=cfg1, L2=cfg2)
    return in_maps, cfg


def _split_gather(nc, qn, gtile, src, idx_tile, ch0, nch, elem):
    """Issue a gather as two half-gathers on different SWDGE queues."""
    h1 = (nch + 1) // 2
    for lo, hi in ((0, h1), (h1, nch)):
        if hi <= lo:
            continue
        ni = (hi - lo) * P
        nc.gpsimd.dma_gather(
            gtile[:, lo:hi, :], src,
            idx_tile[:, (ch0 + lo) * 8:(ch0 + hi) * 8],
            ni, ni, elem, single_packet=False, queue_num=qn(0))


def _build(cfg):
    NPC, NPAD, NBLK = cfg["NPC"], cfg["NPAD"], cfg["NBLK"]
    HI_BASE, H, C = cfg["HI_BASE"], cfg["H"], cfg["C"]
    sgs = cfg["sgs"]
    L1, L2 = cfg["L1"], cfg["L2"]
    AF = mybir.ActivationFunctionType
    AL = mybir.AluOpType

    nc = bacc.Bacc("TRN2", target_bir_lowering=False, debug=False,
                   num_devices=NCORES, num_swdge_queues=4)

    xtab_d = nc.dram_tensor("xtab", [NPAD, P], F16, kind="ExternalInput")
    xloc_d = nc.dram_tensor("xloc", [NPC, P], F16, kind="ExternalInput")
    S1_d = nc.dram_tensor("S1_T", [P, L1["SCHT"] * P], F16,
                          kind="ExternalInput")
    S2_d = nc.dram_tensor("S2_T", [P, L2["SCHT"] * P], F16,
                          kind="ExternalInput")
    W1_d = nc.dram_tensor("W1", [P, H], F16, kind="ExternalInput")
    W2_d = nc.dram_tensor("W2", [P, C], F16, kind="ExternalInput")
    b1_d = nc.dram_tensor("b1c", [P, 1], F32, kind="ExternalInput")
    b2_d = nc.dram_tensor("b2r", [1, C], F16, kind="ExternalInput")
    on_d = nc.dram_tensor("onesr", [1, P], F16, kind="ExternalInput")
    i1l_d = nc.dram_tensor("idx1_lo", [P, max(L1["NLO"], 1) * 8], I16,
                           kind="ExternalInput")
    i1h_d = nc.dram_tensor("idx1_hi", [P, max(L1["NHI"], 1) * 8], I16,
                           kind="ExternalInput")
    i2l_d = nc.dram_tensor("idx2_lo", [P, max(L2["NLO"], 1) * 8], I16,
                           kind="ExternalInput")
    i2h_d = nc.dram_tensor("idx2_hi", [P, max(L2["NHI"], 1) * 8], I16,
                           kind="ExternalInput")
    out_d = nc.dram_tensor("out", [NPC, C], F32, kind="ExternalOutput")

    u2_own = nc.dram_tensor("u2_own", [NPC, P], F16)
    u2_tab = nc.dram_tensor("u2_tab", [NPAD, P], F16, addr_space="Shared")

    rg = [list(range(NCORES))]
    _q = [0]

    def qn(_):
        _q[0] = (_q[0] + 1) % 4
        return _q[0]

    with tile.TileContext(nc) as tc:
        with (
            tc.tile_pool(name="const", bufs=1) as cp,
            tc.tile_pool(name="work", bufs=2) as wp,
            tc.tile_pool(name="psum", bufs=2, space="PSUM") as pp,
        ):
            # ---- constants (idx tables first: gathers gate on them) ----
            i1l = cp.tile([P, max(L1["NLO"], 1) * 8], I16)
            nc.sync.dma_start(i1l[:], i1l_d[:, :])
            i1h = cp.tile([P, max(L1["NHI"], 1) * 8], I16)
            nc.sync.dma_start(i1h[:], i1h_d[:, :])
            W1s = cp.tile([P, H], F16)
            nc.sync.dma_start(W1s[:], W1_d[:, :])
            W2s = cp.tile([P, C], F16)
            nc.sync.dma_start(W2s[:], W2_d[:, :])
            b1s = cp.tile([P, 1], F32)
            nc.sync.dma_start(b1s[:], b1_d[:, :])
            b2s = cp.tile([1, C], F16)
            nc.sync.dma_start(b2s[:], b2_d[:, :])
            ones = cp.tile([1, P], F16)
            nc.sync.dma_start(ones[:], on_d[:, :])

            x_lo = xtab_d[0:LO_LIMIT, :]
            x_hi = xtab_d[HI_BASE:NPAD, :]

            # ---- layer 1: scatter raw x, then W1 / relu / W2 per block ----
            CH_LO, CH_HI = L1["CH_LO"], L1["CH_HI"]
            lo_off, hi_off, soff = L1["lo_off"], L1["hi_off"], L1["soff"]

            def emit_ag(b0, b1_):
                # partial AllGather of a block range's u2 rows into the
                # supergroup-major (contiguous-output) u2_tab layout; the
                # range must cover whole supergroups.
                nsg = b1_ - b0
                go = 8 * b0 * P
                nc.gpsimd.collective_compute(
                    "AllGather", AL.bypass, replica_groups=rg,
                    ins=[u2_own.ap()[b0 * P:b1_ * P, :]],
                    outs=[u2_tab.ap()[go:go + 8 * nsg * P, :]])

            # AG block ranges (whole supergroups) and the sg index at whose
            # loop-top they are emitted: two supergroups after the range
            # completes, so the trigger's wait never stalls gather issue.
            agr = cfg["ag_ranges"]

            for i_sg, (b0, b1_) in enumerate(sgs):
                nlo = lo_off[b1_] - lo_off[b0]
                nhi = hi_off[b1_] - hi_off[b0]
                nst = soff[b1_] - soff[b0]
                if nlo:
                    glo = wp.tile([P, nlo, P], F16, tag="glo", bufs=3)
                    _split_gather(nc, qn, glo, x_lo, i1l, lo_off[b0], nlo, H)
                if nhi:
                    ghi = wp.tile([P, nhi, P], F16, tag="ghi", bufs=3)
                    _split_gather(nc, qn, ghi, x_hi, i1h, hi_off[b0], nhi, H)
                nsg = b1_ - b0
                xsf = wp.tile([P, nsg, P], F16, tag="xsf", bufs=2)
                nc.sync.dma_start(
                    xsf[:],
                    xloc_d.ap().rearrange("(k p) f -> p k f",
                                          p=P)[:, b0:b1_, :])
                sst = wp.tile([P, nst * P], F16, tag="sst", bufs=3)
                nc.sync.dma_start(sst[:],
                                  S1_d[:, soff[b0] * P:soff[b1_] * P])
                for b in range(b0, b1_):
                    sb = (soff[b] - soff[b0]) * P
                    ph = pp.tile([P, P], F32, tag="ph")
                    nc.tensor.matmul(ph[:], xsf[:, b - b0, :],
                                     sst[:, sb:sb + P],
                                     start=True, stop=False)
                    nch = CH_LO[b] + CH_HI[b]
                    for j in range(CH_LO[b]):
                        c = sb + (1 + j) * P
                        g = lo_off[b] - lo_off[b0] + j
                        nc.tensor.matmul(ph[:], glo[:, g, :],
                                         sst[:, c:c + P],
                                         start=False, stop=(j == nch - 1))
                    for j in range(CH_HI[b]):
                        c = sb + (1 + CH_LO[b] + j) * P
                        g = hi_off[b] - hi_off[b0] + j
                        nc.tensor.matmul(ph[:], ghi[:, g, :],
                                         sst[:, c:c + P],
                                         start=False,
                                         stop=(CH_LO[b] + j == nch - 1))
                    g1T = wp.tile([P, P], F16, tag="g1T")
                    nc.vector.tensor_copy(g1T[:], ph[:])
                    ph2 = pp.tile([P, P], F32, tag="ph2")
                    nc.tensor.matmul(ph2[:], W1s[:], g1T[:],
                                     start=True, stop=True)
                    h1T = wp.tile([P, P], F16, tag="h1T")
                    nc.scalar.activation(h1T[:], ph2[:], AF.Relu,
                                         bias=b1s[:, 0:1], scale=1.0)
                    pu2 = pp.tile([P, C], F32, tag="pu2")
                    nc.tensor.matmul(pu2[:], h1T[:], W2s[:],
                                     start=True, stop=True)
                    u2b = wp.tile([P, C], F16, tag="u2b")
                    nc.vector.tensor_copy(u2b[:], pu2[:])
                    nc.sync.dma_start(u2_own[b * P:(b + 1) * P, 0:C], u2b[:])
                qn(0)  # rotate queue mapping so lo/hi loads balance
            # All AGs emitted after the last L1 gather issue: each trigger
            # blocks the GpSimd queue until its collective completes, so
            # inside the loop they would stall gather issue.  Here their
            # range-complete waits are already (or nearly) satisfied, the
            # CC chain overlaps the L1 compute tail, and the only work
            # behind them on this queue -- the L2 gathers -- depends on
            # them anyway.
            for r in agr:
                emit_ag(*r)

            # ---- layer 2: scatter u2 rows, + b2 ----
            i2l = cp.tile([P, max(L2["NLO"], 1) * 8], I16)
            nc.sync.dma_start(i2l[:], i2l_d[:, :])
            i2h = cp.tile([P, max(L2["NHI"], 1) * 8], I16)
            nc.sync.dma_start(i2h[:], i2h_d[:, :])
            u_lo = u2_tab[0:LO_LIMIT, :]
            u_hi = u2_tab[HI_BASE:NPAD, :]
            CH_LO, CH_HI = L2["CH_LO"], L2["CH_HI"]
            lo_off, hi_off, soff = L2["lo_off"], L2["hi_off"], L2["soff"]
            for b0, b1_ in sgs:
                nlo = lo_off[b1_] - lo_off[b0]
                nhi = hi_off[b1_] - hi_off[b0]
                nst = soff[b1_] - soff[b0]
                if nlo:
                    glo = wp.tile([P, nlo, P], F16, tag="glo", bufs=3)
                    _split_gather(nc, qn, glo, u_lo, i2l, lo_off[b0], nlo, H)
                if nhi:
                    ghi = wp.tile([P, nhi, P], F16, tag="ghi", bufs=3)
                    _split_gather(nc, qn, ghi, u_hi, i2h, hi_off[b0], nhi, H)
                nsg = b1_ - b0
                usf = wp.tile([P, nsg, C], F16, tag="usf", bufs=2)
                nc.sync.dma_start(
                    usf[:],
                    u2_own.ap().rearrange("(k p) f -> p k f",
                                          p=P)[:, b0:b1_, 0:C])
                sst = wp.tile([P, nst * P], F16, tag="sst", bufs=3)
                nc.sync.dma_start(sst[:],
                                  S2_d[:, soff[b0] * P:soff[b1_] * P])
                for b in range(b0, b1_):
                    sb = (soff[b] - soff[b0]) * P
                    po = pp.tile([P, C], F32, tag="po")
                    nc.tensor.matmul(po[:], sst[:, sb:sb + P],
                                     usf[:, b - b0, :],
                                     start=True, stop=False)
                    for j in range(CH_LO[b]):
                        c = sb + (1 + j) * P
                        g = lo_off[b] - lo_off[b0] + j
                        nc.tensor.matmul(po[:], sst[:, c:c + P],
                                         glo[:, g, 0:C],
                                         start=False, stop=False)
                    for j in range(CH_HI[b]):
                        c = sb + (1 + CH_LO[b] + j) * P
                        g = hi_off[b] - hi_off[b0] + j
                        nc.tensor.matmul(po[:], sst[:, c:c + P],
                                         ghi[:, g, 0:C],
                                         start=False, stop=False)
                    nc.tensor.matmul(po[:], ones[:], b2s[:],
                                     start=False, stop=True)
                    ob = wp.tile([P, C], F32, tag="ob")
                    nc.vector.tensor_copy(ob[:], po[:])
                    nc.sync.dma_start(out_d[b * P:(b + 1) * P, :], ob[:])
                qn(0)  # rotate queue mapping so lo/hi loads balance

    nc.compile()
    return nc


def kernel(x, edge_index, edge_weight, W1, b1, W2, b2):
    in_maps, cfg = _prep(x, edge_index, edge_weight, W1, b1, W2, b2)
    nc = _build(cfg)
    trace = os.environ.get("GCN_TRACE", "0") == "1"
    res = run_bass_kernel_spmd(nc, in_maps, core_ids=list(range(NCORES)),
                               trace=trace)
    _last_results["exec_time_ns"] = res.exec_time_ns
    _last_results["results"] = res
    out = np.concatenate([r["out"] for r in res.results], axis=0)
    return np.ascontiguousarray(out[:cfg["N"]])
